# revision 9
# baseline (speedup 1.0000x reference)
"""GriffinLlamaMLP with top-k masking on 8 trn2 NeuronCores.

Tensor-parallel over the intermediate dim I=11008 -> 1376/core (padded to
1408 = 11*128). Per-token global top-k threshold found by fixed-slope Newton
iterations on AllReduce'd counts; down-proj partials ReduceScattered over
tokens so each core emits a 256-row shard of the output.
"""

import numpy as np
import ml_dtypes

import concourse.bass as bass
import concourse.mybir as mybir
import concourse.tile as tile
from concourse import bacc
from concourse.masks import make_identity

BF16 = mybir.dt.bfloat16
F16 = mybir.dt.float16
F32 = mybir.dt.float32

H = 4096
I_FULL = 11008
K_FRAC = 0.5

# per-core shard geometry
ISH = 1408            # padded shard width (11 * 128)
KK = ISH // 128       # 11 k-blocks for down matmul
KH = H // 128         # 32 k-blocks for gate/up matmuls
CHUNKS = [(0, 512), (512, 512), (1024, 384)]

THR0 = 0.265          # init for per-token threshold (|silu| median)
SLOPE = 1.6e-5        # fixed Newton slope ~ 1/density
NITER = 6


def build_nc(n_cores=8, s_tiles=16, g=4, niter=NITER, i_real_per_core=I_FULL // 8):
    S = s_tiles * 128
    # global target count of zeroed elems incl. per-core zero pads
    k_target = float(int(K_FRAC * i_real_per_core * n_cores)
                     + (ISH - i_real_per_core) * n_cores)
    s_out = S // n_cores

    nc = bacc.Bacc("TRN2", target_bir_lowering=False, debug=False,
                   enable_asserts=False, num_devices=n_cores)

    xt = nc.dram_tensor("xt", [KH, 128, S], BF16, kind="ExternalInput")
    wg = nc.dram_tensor("wg", [KH, 128, ISH], BF16, kind="ExternalInput")
    wu = nc.dram_tensor("wu", [KH, 128, ISH], BF16, kind="ExternalInput")
    wd = nc.dram_tensor("wd", [KK, 128, H], BF16, kind="ExternalInput")
    out = nc.dram_tensor("out", [s_out, H], F32, kind="ExternalOutput")

    shared = "Shared" if n_cores > 4 else "Local"
    partial = nc.dram_tensor("partial", [S, H], F32, kind="Internal")
    rsout = nc.dram_tensor("rsout", [s_out, H], F32, kind="Internal")
    cins = [nc.dram_tensor(f"cin{i}", [128, s_tiles], F32, kind="Internal")
            for i in range(niter)]
    couts = [nc.dram_tensor(f"cout{i}", [128, s_tiles], F32, kind="Internal",
                            addr_space=shared) for i in range(niter)]
    rg = [list(range(n_cores))]

    with tile.TileContext(nc) as tc:
        with (
            tc.tile_pool(name="persist", bufs=1) as pp,
            tc.tile_pool(name="small", bufs=1) as sp,
            tc.tile_pool(name="ind", bufs=2) as indp,
        ):
            p_sb = pp.tile([128, s_tiles, ISH], BF16, tag="p")
            a_sb = pp.tile([128, s_tiles, ISH], F16, tag="a")
            ident = sp.tile([128, 128], BF16, tag="ident")
            make_identity(nc, ident)
            thr = sp.tile([128, s_tiles], F32, tag="thr")
            cnt = sp.tile([128, s_tiles], F32, tag="cnt")
            cntg = sp.tile([128, s_tiles], F32, tag="cntg")
            delta = sp.tile([128, s_tiles], F32, tag="delta")
            nc.vector.memset(thr, THR0)

            # ---------------- Phase A: gate/up matmuls, silu, |s|, p = s*u
            with (
                tc.tile_pool(name="wpool", bufs=1) as wpool,
                tc.tile_pool(name="xpool", bufs=6) as xpool,
                tc.tile_pool(name="spool", bufs=3) as spool,
                tc.tile_pool(name="psA", bufs=1, space="PSUM") as psA,
            ):
                for (c0, cw) in CHUNKS:
                    wg_c = wpool.tile([128, KH, cw], BF16, tag="wg")
                    wu_c = wpool.tile([128, KH, cw], BF16, tag="wu")
                    for k in range(KH):
                        nc.sync.dma_start(out=wg_c[:, k, :], in_=wg[k, :, c0:c0 + cw])
                        nc.sync.dma_start(out=wu_c[:, k, :], in_=wu[k, :, c0:c0 + cw])
                    for sg in range(s_tiles // g):
                        pgs = [psA.tile([128, cw], F32, tag=f"pg{t}", name=f"pg{t}") for t in range(g)]
                        pus = [psA.tile([128, cw], F32, tag=f"pu{t}", name=f"pu{t}") for t in range(g)]
                        for k in range(KH):
                            xk = xpool.tile([128, g * 128], BF16, tag="x")
                            nc.sync.dma_start(
                                out=xk, in_=xt[k, :, sg * g * 128:(sg + 1) * g * 128])
                            for t in range(g):
                                lhsT = xk[:, t * 128:(t + 1) * 128]
                                nc.tensor.matmul(pgs[t], lhsT, wg_c[:, k, :],
                                                 start=(k == 0), stop=(k == KH - 1))
                                nc.tensor.matmul(pus[t], lhsT, wu_c[:, k, :],
                                                 start=(k == 0), stop=(k == KH - 1))
                        for t in range(g):
                            j = sg * g + t
                            sg_t = spool.tile([128, cw], F16, tag="sg")
                            st = spool.tile([128, cw], F16, tag="s")
                            nc.scalar.activation(
                                sg_t, pgs[t], mybir.ActivationFunctionType.Sigmoid)
                            nc.vector.tensor_mul(st, sg_t, pgs[t])
                            nc.scalar.activation(
                                a_sb[:, j, c0:c0 + cw], st,
                                mybir.ActivationFunctionType.Abs)
                            nc.vector.tensor_mul(p_sb[:, j, c0:c0 + cw], st, pus[t])

            # wd loads overlap phase B
            with tc.tile_pool(name="wdpool", bufs=1) as wdpool:
                wd_sb = wdpool.tile([128, KK, H], BF16, tag="wd")
                for kk in range(KK):
                    for n4 in range(4):
                        nc.sync.dma_start(
                            out=wd_sb[:, kk, n4 * 1024:(n4 + 1) * 1024],
                            in_=wd[kk, :, n4 * 1024:(n4 + 1) * 1024])

                # ------------ Phase B: Newton iterations on global counts
                for it in range(niter):
                    for j in range(s_tiles):
                        scr = indp.tile([128, ISH], BF16, tag="ind")
                        nc.vector.tensor_scalar(
                            out=scr, in0=a_sb[:, j, :], scalar1=thr[:, j:j + 1],
                            scalar2=None, op0=mybir.AluOpType.is_lt,
                            op1=mybir.AluOpType.add,
                            accum_out=cnt[:, j:j + 1])
                    nc.sync.dma_start(out=cins[it][:, :], in_=cnt)
                    nc.gpsimd.collective_compute(
                        "AllReduce", mybir.AluOpType.add, replica_groups=rg,
                        ins=[cins[it].ap().opt()], outs=[couts[it].ap().opt()])
                    nc.sync.dma_start(out=cntg, in_=couts[it][:, :])
                    # thr -= (cnt_global - k_target) * SLOPE, then clamp
                    nc.vector.tensor_scalar(
                        out=delta, in0=cntg, scalar1=k_target, scalar2=SLOPE,
                        op0=mybir.AluOpType.subtract, op1=mybir.AluOpType.mult)
                    nc.vector.tensor_sub(thr, thr, delta)
                    nc.vector.tensor_scalar_max(thr, thr, 0.12)
                    nc.vector.tensor_scalar_min(thr, thr, 0.42)

                # ------------ Phase C: mask, transpose, down matmul
                with (
                    tc.tile_pool(name="s2tp", bufs=2) as s2tp,
                    tc.tile_pool(name="obp", bufs=3) as obp,
                    tc.tile_pool(name="ptp", bufs=2, space="PSUM") as ptp,
                    tc.tile_pool(name="pdp", bufs=2, space="PSUM") as pdp,
                ):
                    for j in range(s_tiles):
                        # p[j] *= (a[j] >= thr_j)
                        nc.vector.scalar_tensor_tensor(
                            out=p_sb[:, j, :], in0=a_sb[:, j, :],
                            scalar=thr[:, j:j + 1], in1=p_sb[:, j, :],
                            op0=mybir.AluOpType.is_ge, op1=mybir.AluOpType.mult)
                        s2t = s2tp.tile([128, KK, 128], BF16, tag="s2t")
                        for kk in range(KK):
                            pt = ptp.tile([128, 128], BF16, tag="pt")
                            nc.tensor.transpose(
                                pt, p_sb[:, j, kk * 128:(kk + 1) * 128], ident)
                            nc.vector.tensor_copy(s2t[:, kk, :], pt)
                        for q in range(4):
                            pd = pdp.tile([128, 1024], F32, tag="pd")
                            for kk in range(KK):
                                nc.tensor.matmul(
                                    pd[:, 0:512], s2t[:, kk, :],
                                    wd_sb[:, kk, q * 1024:q * 1024 + 512],
                                    start=(kk == 0), stop=(kk == KK - 1))
                                nc.tensor.matmul(
                                    pd[:, 512:1024], s2t[:, kk, :],
                                    wd_sb[:, kk, q * 1024 + 512:(q + 1) * 1024],
                                    start=(kk == 0), stop=(kk == KK - 1))
                            ob = obp.tile([128, 1024], F32, tag="ob")
                            nc.vector.tensor_copy(ob, pd)
                            nc.sync.dma_start(
                                out=partial[j * 128:(j + 1) * 128,
                                            q * 1024:(q + 1) * 1024],
                                in_=ob)

            nc.gpsimd.collective_compute(
                "ReduceScatter", mybir.AluOpType.add, replica_groups=rg,
                ins=[partial.ap().opt()], outs=[rsout.ap().opt()])
            nc.sync.dma_start(out=out[:, :], in_=rsout[:, :])

    nc.compile()
    return nc


def _bf16(a):
    return np.ascontiguousarray(a.astype(ml_dtypes.bfloat16))


def prep_inputs(x, w_gate, w_up, w_down, n_cores=8, s_tiles=16,
                i_real_per_core=I_FULL // 8):
    """Host-side shard + transpose + bf16 cast. Returns in_maps for spmd run."""
    S = s_tiles * 128
    x2 = np.asarray(x, np.float32).reshape(S, H)
    xt = _bf16(x2.T.reshape(KH, 128, S))
    in_maps = []
    for c in range(n_cores):
        lo = c * i_real_per_core
        hi = lo + i_real_per_core
        wgs = np.zeros((H, ISH), np.float32)
        wus = np.zeros((H, ISH), np.float32)
        wds = np.zeros((ISH, H), np.float32)
        wgs[:, :i_real_per_core] = np.asarray(w_gate, np.float32)[lo:hi, :].T
        wus[:, :i_real_per_core] = np.asarray(w_up, np.float32)[lo:hi, :].T
        wds[:i_real_per_core, :] = np.asarray(w_down, np.float32)[:, lo:hi].T
        in_maps.append({
            "xt": xt,
            "wg": _bf16(wgs.reshape(KH, 128, ISH)),
            "wu": _bf16(wus.reshape(KH, 128, ISH)),
            "wd": _bf16(wds.reshape(KK, 128, H)),
        })
    return in_maps


_NC_CACHE = {}


def kernel(x, w_gate, w_up, w_down):
    from concourse.bass_utils import run_bass_kernel_spmd
    n_cores = 8
    s_tiles = 16
    key = (n_cores, s_tiles)
    if key not in _NC_CACHE:
        _NC_CACHE[key] = build_nc(n_cores=n_cores, s_tiles=s_tiles)
    nc = _NC_CACHE[key]
    in_maps = prep_inputs(x, w_gate, w_up, w_down, n_cores, s_tiles)
    res = run_bass_kernel_spmd(nc, in_maps, core_ids=list(range(n_cores)))
    shards = [res.results[c]["out"] for c in range(n_cores)]
    full = np.concatenate(shards, axis=0).astype(np.float32)
    return full.reshape(1, s_tiles * 128, H)


# revision 10
# speedup vs baseline: 1.2231x; 1.2231x over previous
"""GriffinLlamaMLP with top-k masking on 8 trn2 NeuronCores.

Tensor-parallel over the intermediate dim I=11008 -> 1376/core (padded to
1408 = 11*128). Per-token global top-k threshold found by fixed-slope Newton
iterations on AllReduce'd counts; down-proj partials ReduceScattered over
tokens so each core emits a 256-row shard of the output.
"""

import numpy as np
import ml_dtypes

import concourse.bass as bass
import concourse.mybir as mybir
import concourse.tile as tile
from concourse import bacc
from concourse.masks import make_identity

BF16 = mybir.dt.bfloat16
F16 = mybir.dt.float16
F32 = mybir.dt.float32

H = 4096
I_FULL = 11008
K_FRAC = 0.5

# per-core shard geometry
ISH = 1408            # padded shard width (11 * 128)
KK = ISH // 128       # 11 k-blocks for down matmul
KH = H // 128         # 32 k-blocks for gate/up matmuls
CHUNKS = [(0, 512), (512, 512), (1024, 384)]

THR0 = 0.265          # init for per-token threshold (|silu| median)
SLOPE = 1.6e-5        # fixed Newton slope ~ 1/density
NITER = 4


def build_nc(n_cores=8, s_tiles=16, g=4, niter=NITER, i_real_per_core=I_FULL // 8):
    S = s_tiles * 128
    # global target count of zeroed elems incl. per-core zero pads
    k_target = float(int(K_FRAC * i_real_per_core * n_cores)
                     + (ISH - i_real_per_core) * n_cores)
    s_out = S // n_cores

    nc = bacc.Bacc("TRN2", target_bir_lowering=False, debug=False,
                   enable_asserts=False, num_devices=n_cores)

    xt = nc.dram_tensor("xt", [KH, 128, S], BF16, kind="ExternalInput")
    wg = nc.dram_tensor("wg", [KH, 128, ISH], BF16, kind="ExternalInput")
    wu = nc.dram_tensor("wu", [KH, 128, ISH], BF16, kind="ExternalInput")
    wd = nc.dram_tensor("wd", [KK, 128, H], BF16, kind="ExternalInput")
    out = nc.dram_tensor("out", [s_out, H], F32, kind="ExternalOutput")

    shared = "Shared" if n_cores > 4 else "Local"
    partial = nc.dram_tensor("partial", [S, H], F32, kind="Internal")
    rsout = nc.dram_tensor("rsout", [s_out, H], F32, kind="Internal")
    cins = [nc.dram_tensor(f"cin{i}", [128, s_tiles], F32, kind="Internal")
            for i in range(niter)]
    couts = [nc.dram_tensor(f"cout{i}", [128, s_tiles], F32, kind="Internal",
                            addr_space=shared) for i in range(niter)]
    rg = [list(range(n_cores))]

    with tile.TileContext(nc) as tc:
        with (
            tc.tile_pool(name="persist", bufs=1) as pp,
            tc.tile_pool(name="small", bufs=1) as sp,
            tc.tile_pool(name="ind", bufs=2) as indp,
        ):
            p_sb = pp.tile([128, s_tiles, ISH], BF16, tag="p")
            a_sb = pp.tile([128, s_tiles, ISH], F16, tag="a")
            ident = sp.tile([128, 128], BF16, tag="ident")
            make_identity(nc, ident)
            thr = sp.tile([128, s_tiles], F32, tag="thr")
            cnt = sp.tile([128, s_tiles], F32, tag="cnt")
            cntg = sp.tile([128, s_tiles], F32, tag="cntg")
            delta = sp.tile([128, s_tiles], F32, tag="delta")
            nc.vector.memset(thr, THR0)

            # ---------------- Phase A: gate/up matmuls, silu, |s|, p = s*u
            with (
                tc.tile_pool(name="wpool", bufs=1) as wpool,
                tc.tile_pool(name="xpool", bufs=6) as xpool,
                tc.tile_pool(name="spool", bufs=3) as spool,
                tc.tile_pool(name="psA", bufs=1, space="PSUM") as psA,
            ):
                for (c0, cw) in CHUNKS:
                    wg_c = wpool.tile([128, KH, cw], BF16, tag="wg")
                    wu_c = wpool.tile([128, KH, cw], BF16, tag="wu")
                    for k in range(KH):
                        nc.sync.dma_start(out=wg_c[:, k, :], in_=wg[k, :, c0:c0 + cw])
                        nc.sync.dma_start(out=wu_c[:, k, :], in_=wu[k, :, c0:c0 + cw])
                    for sg in range(s_tiles // g):
                        pgs = [psA.tile([128, cw], F32, tag=f"pg{t}", name=f"pg{t}") for t in range(g)]
                        pus = [psA.tile([128, cw], F32, tag=f"pu{t}", name=f"pu{t}") for t in range(g)]
                        for k in range(KH):
                            xk = xpool.tile([128, g * 128], BF16, tag="x")
                            nc.sync.dma_start(
                                out=xk, in_=xt[k, :, sg * g * 128:(sg + 1) * g * 128])
                            for t in range(g):
                                lhsT = xk[:, t * 128:(t + 1) * 128]
                                nc.tensor.matmul(pgs[t], lhsT, wg_c[:, k, :],
                                                 start=(k == 0), stop=(k == KH - 1))
                                nc.tensor.matmul(pus[t], lhsT, wu_c[:, k, :],
                                                 start=(k == 0), stop=(k == KH - 1))
                        for t in range(g):
                            j = sg * g + t
                            sg_t = spool.tile([128, cw], F16, tag="sg")
                            st = spool.tile([128, cw], F16, tag="s")
                            nc.scalar.activation(
                                sg_t, pgs[t], mybir.ActivationFunctionType.Sigmoid)
                            nc.vector.tensor_mul(st, sg_t, pgs[t])
                            nc.scalar.activation(
                                a_sb[:, j, c0:c0 + cw], st,
                                mybir.ActivationFunctionType.Abs)
                            nc.vector.tensor_mul(p_sb[:, j, c0:c0 + cw], st, pus[t])

            # wd loads overlap phase B
            with tc.tile_pool(name="wdpool", bufs=1) as wdpool:
                wd_sb = wdpool.tile([128, KK, H], BF16, tag="wd")
                for kk in range(KK):
                    for n4 in range(4):
                        nc.sync.dma_start(
                            out=wd_sb[:, kk, n4 * 1024:(n4 + 1) * 1024],
                            in_=wd[kk, :, n4 * 1024:(n4 + 1) * 1024])

                # ------------ Phase B: Newton iterations on global counts
                for it in range(niter):
                    for j in range(s_tiles):
                        scr = indp.tile([128, ISH], BF16, tag="ind")
                        nc.vector.tensor_scalar(
                            out=scr, in0=a_sb[:, j, :], scalar1=thr[:, j:j + 1],
                            scalar2=None, op0=mybir.AluOpType.is_lt,
                            op1=mybir.AluOpType.add,
                            accum_out=cnt[:, j:j + 1])
                    nc.sync.dma_start(out=cins[it][:, :], in_=cnt)
                    nc.gpsimd.collective_compute(
                        "AllReduce", mybir.AluOpType.add, replica_groups=rg,
                        ins=[cins[it].ap().opt()], outs=[couts[it].ap().opt()])
                    nc.sync.dma_start(out=cntg, in_=couts[it][:, :])
                    # thr -= (cnt_global - k_target) * SLOPE, then clamp
                    nc.vector.tensor_scalar(
                        out=delta, in0=cntg, scalar1=k_target, scalar2=SLOPE,
                        op0=mybir.AluOpType.subtract, op1=mybir.AluOpType.mult)
                    nc.vector.tensor_sub(thr, thr, delta)
                    nc.vector.tensor_scalar_max(thr, thr, 0.12)
                    nc.vector.tensor_scalar_min(thr, thr, 0.42)

                # ------------ Phase C: mask, transpose, down matmul
                with (
                    tc.tile_pool(name="s2tp", bufs=2) as s2tp,
                    tc.tile_pool(name="obp", bufs=3) as obp,
                    tc.tile_pool(name="ptp", bufs=2, space="PSUM") as ptp,
                    tc.tile_pool(name="pdp", bufs=2, space="PSUM") as pdp,
                ):
                    for j in range(s_tiles):
                        # p[j] *= (a[j] >= thr_j)
                        nc.vector.scalar_tensor_tensor(
                            out=p_sb[:, j, :], in0=a_sb[:, j, :],
                            scalar=thr[:, j:j + 1], in1=p_sb[:, j, :],
                            op0=mybir.AluOpType.is_ge, op1=mybir.AluOpType.mult)
                        s2t = s2tp.tile([128, KK, 128], BF16, tag="s2t")
                        for kk in range(KK):
                            pt = ptp.tile([128, 128], BF16, tag="pt")
                            nc.tensor.transpose(
                                pt, p_sb[:, j, kk * 128:(kk + 1) * 128], ident)
                            nc.vector.tensor_copy(s2t[:, kk, :], pt)
                        for q in range(4):
                            pd = pdp.tile([128, 1024], F32, tag="pd")
                            for kk in range(KK):
                                nc.tensor.matmul(
                                    pd[:, 0:512], s2t[:, kk, :],
                                    wd_sb[:, kk, q * 1024:q * 1024 + 512],
                                    start=(kk == 0), stop=(kk == KK - 1))
                                nc.tensor.matmul(
                                    pd[:, 512:1024], s2t[:, kk, :],
                                    wd_sb[:, kk, q * 1024 + 512:(q + 1) * 1024],
                                    start=(kk == 0), stop=(kk == KK - 1))
                            ob = obp.tile([128, 1024], F32, tag="ob")
                            nc.vector.tensor_copy(ob, pd)
                            nc.sync.dma_start(
                                out=partial[j * 128:(j + 1) * 128,
                                            q * 1024:(q + 1) * 1024],
                                in_=ob)

            nc.gpsimd.collective_compute(
                "ReduceScatter", mybir.AluOpType.add, replica_groups=rg,
                ins=[partial.ap().opt()], outs=[rsout.ap().opt()])
            nc.sync.dma_start(out=out[:, :], in_=rsout[:, :])

    nc.compile()
    return nc


def _bf16(a):
    return np.ascontiguousarray(a.astype(ml_dtypes.bfloat16))


def prep_inputs(x, w_gate, w_up, w_down, n_cores=8, s_tiles=16,
                i_real_per_core=I_FULL // 8):
    """Host-side shard + transpose + bf16 cast. Returns in_maps for spmd run."""
    S = s_tiles * 128
    x2 = np.asarray(x, np.float32).reshape(S, H)
    xt = _bf16(x2.T.reshape(KH, 128, S))
    in_maps = []
    for c in range(n_cores):
        lo = c * i_real_per_core
        hi = lo + i_real_per_core
        wgs = np.zeros((H, ISH), np.float32)
        wus = np.zeros((H, ISH), np.float32)
        wds = np.zeros((ISH, H), np.float32)
        wgs[:, :i_real_per_core] = np.asarray(w_gate, np.float32)[lo:hi, :].T
        wus[:, :i_real_per_core] = np.asarray(w_up, np.float32)[lo:hi, :].T
        wds[:i_real_per_core, :] = np.asarray(w_down, np.float32)[:, lo:hi].T
        in_maps.append({
            "xt": xt,
            "wg": _bf16(wgs.reshape(KH, 128, ISH)),
            "wu": _bf16(wus.reshape(KH, 128, ISH)),
            "wd": _bf16(wds.reshape(KK, 128, H)),
        })
    return in_maps


_NC_CACHE = {}


def kernel(x, w_gate, w_up, w_down):
    from concourse.bass_utils import run_bass_kernel_spmd
    n_cores = 8
    s_tiles = 16
    key = (n_cores, s_tiles)
    if key not in _NC_CACHE:
        _NC_CACHE[key] = build_nc(n_cores=n_cores, s_tiles=s_tiles)
    nc = _NC_CACHE[key]
    in_maps = prep_inputs(x, w_gate, w_up, w_down, n_cores, s_tiles)
    res = run_bass_kernel_spmd(nc, in_maps, core_ids=list(range(n_cores)))
    shards = [res.results[c]["out"] for c in range(n_cores)]
    full = np.concatenate(shards, axis=0).astype(np.float32)
    return full.reshape(1, s_tiles * 128, H)
